# revision 1
# baseline (speedup 1.0000x reference)
"""2-layer GAT (DGL GATConv style) on 8 Trainium2 NeuronCores via Bass.

Sharding: nodes/edges partitioned by dst across 8 cores. Each core:
  P1: computes the full layer-1 projection table T1[ROWS, T1C] (replicated)
      rows: [ft1 (H1*D1) | el1 (H1) | er1 (H1) | pad | ER section: er1 | pad]
  P2: processes its own dst-sorted edges: dma_gather rows of T1 by src
      (messages + el), by dst (er); w = exp(leaky_relu(el+er)); indicator
      matmul scatter-adds w*msg and w into per-window PSUM; then
      out = num/den + b1, ELU -> h'.
  P3: transposes h', computes ft2ext = h' @ [W2|wl2|wr2]; AllGather the
      per-core [NPAD, T2C] blocks into T2[ROWS, T2C].
  P4: same edge machinery for layer 2 (1 head) -> out rows.
Host only partitions/permutes indices and concatenates outputs.
"""

import sys
import numpy as np

sys.path.insert(0, "/opt/trn_rl_repo")

import concourse.bass as bass  # noqa: E402
import concourse.tile as tile  # noqa: E402
from concourse import bacc, mybir  # noqa: E402
from concourse.ap import AP  # noqa: E402
from concourse.bass_utils import run_bass_kernel_spmd  # noqa: E402

F32 = mybir.dt.float32
F32R = mybir.dt.float32r
BF16 = mybir.dt.bfloat16
I16 = mybir.dt.int16
AF = mybir.ActivationFunctionType
ALU = mybir.AluOpType

CORES = 8
NEG_SLOPE = 0.2
# dtype of the gather tables (message payloads); fp32 values for el/er either way
BF1 = True   # layer-1 table in bf16
BF2 = True   # layer-2 table in bf16
CHUNK = 8    # edge tiles per gather chunk (1024 descriptors per gather)


def _cdiv(a, b):
    return (a + b - 1) // b


class Dims:
    def __init__(self, n_nodes, in_size, h1, d1, d2, t_w, chunk, bf1=False, bf2=False):
        self.N = n_nodes
        self.NPC = n_nodes // CORES            # nodes per core (owned)
        assert self.NPC * CORES == n_nodes
        self.NPAD = _cdiv(self.NPC, 128) * 128  # padded nodes per core
        self.W = self.NPAD // 128               # windows per core
        self.ROWS = self.NPAD * CORES           # global padded rows
        self.IN = in_size
        assert in_size % 128 == 0
        self.KC1 = in_size // 128               # k-chunks for layer-1 proj
        self.H1, self.D1, self.D2 = h1, d1, d2
        self.FT1 = h1 * d1                      # 512
        assert self.FT1 % 128 == 0
        self.FT2 = d2
        self.KC2 = self.FT1 // 128              # k-chunks for layer-2 proj (4)
        self.T_w = t_w                          # tiles per window (128 edges each)
        self.T1T = self.W * t_w                 # total edge tiles per core
        self.EPAD = self.T1T * 128              # padded edges per core
        self.C = chunk                          # tiles per gather chunk
        # table geometry in units of the table dtype; L = fp32 lanes
        self.BF1, self.BF2 = bf1, bf2
        self.L1L = 2 if bf1 else 1
        self.L2L = 2 if bf2 else 1
        u1 = 128 if bf1 else 64                 # units per 256B
        u2 = 128 if bf2 else 64
        self.G1E = _cdiv(self.FT1 + h1 * self.L1L, u1) * u1
        self.ER1U = _cdiv(h1 * self.L1L, u1) * u1
        self.T1C = self.G1E + self.ER1U
        self.G2E = _cdiv(self.FT2 + 2 * self.L2L, u2) * u2
        self.ER2U = _cdiv(self.L2L, u2) * u2
        self.T2C = self.G2E + self.ER2U

    def key(self):
        return (self.N, self.IN, self.H1, self.D1, self.D2, self.T_w, self.C,
                self.BF1, self.BF2)


def _v(base: AP, off: int, pairs) -> AP:
    """SBUF view helper: keep the partition dim of `base`, replace free dims."""
    return AP(base.tensor, base.offset + off, [list(base.ap[0])] + [list(p) for p in pairs])


def build_program(dm: Dims, repeat: int = 1, no_collective: bool = False, skip_g: bool = False, skip_er: bool = False):
    nc = bacc.Bacc("TRN2", target_bir_lowering=False, debug=False, num_devices=CORES,
                   num_swdge_queues=4)

    H1, D1, D2, IN = dm.H1, dm.D1, dm.D2, dm.IN
    FT1, FT2 = dm.FT1, dm.FT2

    # ---- I/O ----
    xT = nc.dram_tensor("xT", [IN, dm.ROWS], F32R, kind="ExternalInput")
    W1 = nc.dram_tensor("W1", [IN, FT1], F32R, kind="ExternalInput")
    W1T = nc.dram_tensor("W1T", [FT1, IN], F32, kind="ExternalInput")
    AlB1 = nc.dram_tensor("AlB1", [FT1, H1], F32, kind="ExternalInput")
    ArB1 = nc.dram_tensor("ArB1", [FT1, H1], F32, kind="ExternalInput")
    b1r = nc.dram_tensor("b1r", [1, FT1], F32, kind="ExternalInput")
    W2 = nc.dram_tensor("W2", [FT1, D2], F32R, kind="ExternalInput")
    W2T = nc.dram_tensor("W2T", [D2, FT1], F32, kind="ExternalInput")
    al2c = nc.dram_tensor("al2c", [D2, 1], F32, kind="ExternalInput")
    ar2c = nc.dram_tensor("ar2c", [D2, 1], F32, kind="ExternalInput")
    b2r = nc.dram_tensor("b2r", [1, D2], F32, kind="ExternalInput")
    iotaD = nc.dram_tensor("iota", [128, 128], F32, kind="ExternalInput")
    identD = nc.dram_tensor("ident", [128, 128], F32, kind="ExternalInput")
    srcwD = nc.dram_tensor("srcw", [128, dm.EPAD // 16], I16, kind="ExternalInput")
    dstwD = nc.dram_tensor("dstw", [128, dm.EPAD // 16], I16, kind="ExternalInput")
    dstlD = nc.dram_tensor("dstl", [128, dm.EPAD // 16], I16, kind="ExternalInput")
    drelD = nc.dram_tensor("drel", [128, dm.T1T], F32, kind="ExternalInput")
    outD = nc.dram_tensor("out", [dm.NPAD, D2], F32, kind="ExternalOutput")

    with tile.TileContext(nc) as tc:
        import contextlib
        ctx = contextlib.ExitStack()
        with ctx:
            dram = ctx.enter_context(tc.tile_pool(name="dram", bufs=1, space="DRAM"))
            const = ctx.enter_context(tc.tile_pool(name="const", bufs=1))
            ps_big = ctx.enter_context(tc.tile_pool(name="ps_big", bufs=2, space="PSUM"))
            ps_small = ctx.enter_context(tc.tile_pool(name="ps_small", bufs=2, space="PSUM"))
            ps_med = ctx.enter_context(tc.tile_pool(name="ps_med", bufs=2, space="PSUM"))
            ps_den = ctx.enter_context(tc.tile_pool(name="ps_den", bufs=2, space="PSUM"))
            work = ctx.enter_context(tc.tile_pool(name="work", bufs=2))
            xpool = ctx.enter_context(tc.tile_pool(name="xpool", bufs=2))
            gpool = ctx.enter_context(tc.tile_pool(name="gpool", bufs=2))
            mpool = ctx.enter_context(tc.tile_pool(name="mpool", bufs=2))
            spool = ctx.enter_context(tc.tile_pool(name="spool", bufs=2))
            hpool = ctx.enter_context(tc.tile_pool(name="hpool", bufs=1))

            TD1 = BF16 if dm.BF1 else F32
            TD2 = BF16 if dm.BF2 else F32
            sz1 = 2 if dm.BF1 else 4
            sz2 = 2 if dm.BF2 else 4
            T1 = dram.tile([dm.ROWS, dm.T1C], TD1)
            AGin = dram.tile([dm.NPAD, dm.G2E], TD2)
            ER2loc = dram.tile([dm.NPAD, dm.ER2U], TD2)
            T2 = dram.tile([dm.ROWS, dm.G2E], TD2)

            # ---- resident constants ----
            iota_sb = const.tile([128, 128], F32)
            nc.sync.dma_start(iota_sb[:], iotaD.ap())
            ident_sb = const.tile([128, 128], F32)
            nc.sync.dma_start(ident_sb[:], identD.ap())
            srcw_sb = const.tile([128, dm.EPAD // 16], I16)
            nc.sync.dma_start(srcw_sb[:], srcwD.ap())
            dstw_sb = const.tile([128, dm.EPAD // 16], I16)
            nc.sync.dma_start(dstw_sb[:], dstwD.ap())
            dstl_sb = const.tile([128, dm.EPAD // 16], I16)
            nc.sync.dma_start(dstl_sb[:], dstlD.ap())
            drel_sb = const.tile([128, dm.T1T], F32)
            nc.sync.dma_start(drel_sb[:], drelD.ap())

            # W1ext chunks: [128, FT1 + 2*H1] x KC1
            w1ext = []
            for kc in range(dm.KC1):
                t = const.tile([128, FT1 + 2 * H1], F32R, tag=f"w1ext{kc}", name=f"w1ext{kc}")
                nc.sync.dma_start(t[:, 0:FT1], W1[kc * 128:(kc + 1) * 128, :])
                w1ext.append(t)
            # wl1 / wr1 via W1T @ AlB1 (full fp32 matmuls, one-time)
            w1t_sb = []
            for kc2 in range(dm.KC2):
                for m in range(dm.KC1):
                    t = work.tile([128, 128], F32, tag=f"w1t{kc2}_{m}", name=f"w1t{kc2}_{m}")
                    nc.sync.dma_start(t[:], W1T[kc2 * 128:(kc2 + 1) * 128, m * 128:(m + 1) * 128])
                    w1t_sb.append(t)
            alb_sb = []
            for kc2 in range(dm.KC2):
                t = work.tile([128, 2 * H1], F32, tag=f"albl{kc2}", name=f"albl{kc2}")
                nc.sync.dma_start(t[:, 0:H1], AlB1[kc2 * 128:(kc2 + 1) * 128, :])
                nc.sync.dma_start(t[:, H1:2 * H1], ArB1[kc2 * 128:(kc2 + 1) * 128, :])
                alb_sb.append(t)
            for m in range(dm.KC1):
                pw = ps_small.tile([128, 2 * H1], F32, tag="pssm")
                for kc2 in range(dm.KC2):
                    nc.tensor.matmul(pw[:], w1t_sb[kc2 * dm.KC1 + m][:], alb_sb[kc2][:],
                                     start=(kc2 == 0), stop=(kc2 == dm.KC2 - 1))
                nc.vector.tensor_copy(w1ext[m][:, FT1:FT1 + 2 * H1], pw[:])

            # W2ext chunks: [128, FT2 + 2] x KC2
            w2ext = []
            for kc in range(dm.KC2):
                t = const.tile([128, FT2 + 2], F32R, tag=f"w2ext{kc}", name=f"w2ext{kc}")
                nc.sync.dma_start(t[:, 0:FT2], W2[kc * 128:(kc + 1) * 128, :])
                w2ext.append(t)
            w2t_sb = []
            for m in range(dm.KC2):
                t = work.tile([D2, 128], F32, tag=f"w2t{m}", name=f"w2t{m}")
                nc.sync.dma_start(t[:], W2T[:, m * 128:(m + 1) * 128])
                w2t_sb.append(t)
            al2_sb = work.tile([D2, 2], F32, tag="al2")
            nc.sync.dma_start(al2_sb[:, 0:1], al2c.ap())
            nc.sync.dma_start(al2_sb[:, 1:2], ar2c.ap())
            for m in range(dm.KC2):
                pw = ps_small.tile([128, 2], F32, tag="pssm")
                nc.tensor.matmul(pw[:], w2t_sb[m][:], al2_sb[:], start=True, stop=True)
                nc.vector.tensor_copy(w2ext[m][:, FT2:FT2 + 2], pw[:])

            # bias matrices via rank-1 matmul (ones[1,128]^T @ brow[1,F])
            ones_sb = const.tile([1, 128], F32)
            nc.vector.memset(ones_sb[:], 1.0)
            b1row_sb = work.tile([1, FT1], F32, tag="b1row")
            nc.sync.dma_start(b1row_sb[:], b1r.ap())
            b2row_sb = work.tile([1, D2], F32, tag="b2row")
            nc.sync.dma_start(b2row_sb[:], b2r.ap())
            B1 = const.tile([128, FT1], F32)
            pb = ps_big.tile([128, FT1], F32, tag="psbig")
            nc.tensor.matmul(pb[:], ones_sb[:], b1row_sb[:], start=True, stop=True)
            nc.vector.tensor_copy(B1[:], pb[:])
            B2 = const.tile([128, D2], F32)
            pb2 = ps_med.tile([128, D2], F32, tag="psmed")
            nc.tensor.matmul(pb2[:], ones_sb[:], b2row_sb[:], start=True, stop=True)
            nc.vector.tensor_copy(B2[:], pb2[:])

            for _rep in range(repeat):
                # ---- P1: projection of all ROWS into T1 ----
                n_tiles = dm.ROWS // 128
                SUP = 10  # node-tiles per xT super-load
                n_sup = _cdiv(n_tiles, SUP)
                for sup in range(n_sup):
                    t0 = sup * SUP
                    t1 = min(t0 + SUP, n_tiles)
                    cols = (t1 - t0) * 128
                    xts = []
                    for kc in range(dm.KC1):
                        t = xpool.tile([128, SUP * 128], F32R, tag=f"xsup{kc}", name=f"xsup{sup}_{kc}")
                        nc.sync.dma_start(t[:, 0:cols], xT[kc * 128:(kc + 1) * 128, t0 * 128:t1 * 128])
                        xts.append(t)
                    for nt in range(t0, t1):
                        co = (nt - t0) * 128
                        pA = ps_big.tile([128, FT1], F32, tag="psbig")
                        pB = ps_small.tile([128, 2 * H1], F32, tag="pssm")
                        for kc in range(dm.KC1):
                            lhsT = xts[kc][:, co:co + 128]
                            nc.tensor.matmul(pA[:], lhsT, w1ext[kc][:, 0:FT1],
                                             start=(kc == 0), stop=(kc == dm.KC1 - 1))
                            nc.tensor.matmul(pB[:], lhsT, w1ext[kc][:, FT1:FT1 + 2 * H1],
                                             start=(kc == 0), stop=(kc == dm.KC1 - 1))
                        st = spool.tile([128, dm.T1C], TD1, tag="st1")
                        nc.vector.tensor_copy(st[:, 0:FT1], pA[:])
                        stf = st[:].bitcast(F32)
                        nc.vector.tensor_copy(_v(stf, FT1 * sz1 // 4, [[1, H1]]), pB[:, 0:H1])
                        nc.vector.tensor_copy(_v(stf, dm.G1E * sz1 // 4, [[1, H1]]), pB[:, H1:2 * H1])
                        nc.sync.dma_start(T1[nt * 128:(nt + 1) * 128, :], st[:])

                # ---- P2: layer-1 edge pass ----
                h_sb = []  # per-window h' tiles
                for w in range(dm.W):
                    h_sb.append(hpool.tile([128, FT1], F32, tag=f"hwin{w}", name=f"hwin{w}"))

                def edge_pass(tab, tabC, gE, erTab, erTabC, erOff, erU, erIdx,
                              ftC, nH, w1_is_l1, TD, szt):
                    """Shared edge-pass body for both layers. Units = table-dtype elems."""
                    n_chunks = _cdiv(dm.T1T, dm.C)
                    gf = gE * szt // 4
                    ef = erU * szt // 4
                    elo = ftC * szt // 4
                    for ci in range(n_chunks):
                        g0 = ci * dm.C
                        g1 = min(g0 + dm.C, dm.T1T)
                        nt = g1 - g0
                        ni = nt * 128
                        # gather messages (+el) by src
                        gc = gpool.tile([128, dm.C * gE], TD, tag="gmsg" if w1_is_l1 else "gmsg2")
                        if not skip_g:
                            nc.gpsimd.dma_gather(
                                _v(gc[:], 0, [[gE, nt], [1, gE]]),
                                tab[:, 0:gE],
                                srcw_sb[:, g0 * 8:g1 * 8],
                                ni, ni, gE, elem_step=tabC,
                                queue_num=(ci % 2) * 2)
                        else:
                            nc.vector.memset(_v(gc[:], 0, [[1, 4]]), 0.5)
                        # gather er by dst
                        ec = gpool.tile([128, dm.C * erU], TD, tag="ger" if w1_is_l1 else "ger2")
                        if not skip_er:
                            nc.gpsimd.dma_gather(
                                _v(ec[:], 0, [[erU, nt], [1, erU]]),
                                erTab[:, erOff:erOff + erU],
                                erIdx[:, g0 * 8:g1 * 8],
                                ni, ni, erU, elem_step=erTabC,
                                queue_num=1 + (ci % 2) * 2)
                        else:
                            nc.vector.memset(_v(ec[:], 0, [[1, 4]]), 0.5)
                        gcf = gc[:].bitcast(F32)
                        ecf = ec[:].bitcast(F32)
                        # e = el + er ; w = exp(leaky(e))
                        et = work.tile([128, dm.C * nH], F32, tag="et" if w1_is_l1 else "et2")
                        nc.vector.tensor_tensor(
                            _v(et[:], 0, [[nH, nt], [1, nH]]),
                            _v(gcf, elo, [[gf, nt], [1, nH]]),
                            _v(ecf, 0, [[ef, nt], [1, nH]]), ALU.add)
                        lt = work.tile([128, dm.C * nH], F32, tag="lt" if w1_is_l1 else "lt2")
                        nc.scalar.activation(_v(lt[:], 0, [[1, nt * nH]]),
                                             _v(et[:], 0, [[1, nt * nH]]),
                                             AF.Prelu, alpha=NEG_SLOPE)
                        wt = work.tile([128, dm.C * nH], F32, tag="wt" if w1_is_l1 else "wt2")
                        nc.scalar.activation(_v(wt[:], 0, [[1, nt * nH]]),
                                             _v(lt[:], 0, [[1, nt * nH]]),
                                             AF.Exp)
                        # msg = gathered_ft * w (broadcast over D)
                        Dh = ftC // nH
                        nD = nH if nH % 2 == 0 else 2 * nH  # even denom cols for fp32r
                        mrow = ftC + nD
                        mc = mpool.tile([128, dm.C * mrow], F32R,
                                        tag="mc" if w1_is_l1 else "mc2")
                        nc.vector.tensor_tensor(
                            _v(mc[:], 0, [[mrow, nt], [Dh, nH], [1, Dh]]),
                            _v(gc[:], 0, [[gE, nt], [Dh, nH], [1, Dh]]),
                            _v(wt[:], 0, [[nH, nt], [1, nH], [0, Dh]]), ALU.mult)
                        # append w as the denominator column(s), duplicated if odd
                        nc.vector.tensor_copy(
                            _v(mc[:], ftC, [[mrow, nt], [nH, nD // nH], [1, nH]]),
                            _v(wt[:], 0, [[nH, nt], [0, nD // nH], [1, nH]]))
                        for t in range(nt):
                            gt = g0 + t
                            w = gt // dm.T_w
                            first = (gt % dm.T_w == 0)
                            last = (gt % dm.T_w == dm.T_w - 1)
                            ind = work.tile([128, 128], F32R, tag="ind" if w1_is_l1 else "ind2")
                            nc.vector.tensor_scalar(
                                ind[:], iota_sb[:], drel_sb[:, gt:gt + 1], None, ALU.is_equal)
                            if w1_is_l1:
                                if first:
                                    edge_pass.pA = ps_big.tile([128, FT1], F32, tag="psbig", name=f"pa{gt}")
                                    edge_pass.pD = ps_den.tile([128, nH], F32, tag="psden", name=f"pd{gt}")
                                nc.tensor.matmul(
                                    edge_pass.pA[:], ind[:],
                                    _v(mc[:], t * mrow, [[1, ftC]]),
                                    start=first, stop=last)
                                nc.tensor.matmul(
                                    edge_pass.pD[:], ind[:],
                                    _v(mc[:], t * mrow + ftC, [[1, nH]]),
                                    start=first, stop=last)
                                if last:
                                    _evict_l1(w, edge_pass.pA, edge_pass.pD)
                            else:
                                if first:
                                    edge_pass.pM = ps_med.tile([128, mrow], F32, tag="psmed", name=f"pm{gt}")
                                nc.tensor.matmul(
                                    edge_pass.pM[:], ind[:],
                                    _v(mc[:], t * mrow, [[1, mrow]]),
                                    start=first, stop=last)
                                if last:
                                    _evict_l2(w, edge_pass.pM)

                def _evict_l1(w, pA, pD):
                    dr = work.tile([128, H1], F32, tag="dr")
                    nc.vector.tensor_scalar(dr[:], pD[:], 1e-30, None, ALU.add)
                    dri = work.tile([128, H1], F32, tag="dri")
                    nc.vector.reciprocal(dri[:], dr[:])
                    x = work.tile([128, FT1], F32, tag="evx")
                    nc.vector.tensor_tensor(
                        _v(x[:], 0, [[D1, H1], [1, D1]]),
                        _v(pA[:], 0, [[D1, H1], [1, D1]]),
                        _v(dri[:], 0, [[1, H1], [0, D1]]), ALU.mult)
                    nc.vector.tensor_add(x[:], x[:], B1[:])
                    # elu(x) = max(x,0) + exp(min(x,0)) - 1
                    mn = work.tile([128, FT1], F32, tag="evmn")
                    nc.vector.tensor_scalar(mn[:], x[:], 0.0, None, ALU.min)
                    ex = work.tile([128, FT1], F32, tag="evex")
                    nc.scalar.activation(ex[:], mn[:], AF.Exp)
                    mx = work.tile([128, FT1], F32, tag="evmx")
                    nc.vector.tensor_scalar(mx[:], x[:], 0.0, None, ALU.max)
                    h = h_sb[w]
                    nc.vector.tensor_add(h[:], mx[:], ex[:])
                    nc.vector.tensor_scalar(h[:], h[:], -1.0, None, ALU.add)

                def _evict_l2(w, pM):
                    dr = work.tile([128, 1], F32, tag="dr2")
                    nc.vector.tensor_scalar(dr[:], pM[:, FT2:FT2 + 1], 1e-30, None, ALU.add)
                    dri = work.tile([128, 1], F32, tag="dri2")
                    nc.vector.reciprocal(dri[:], dr[:])
                    o = work.tile([128, D2], F32, tag="evo")
                    nc.vector.tensor_tensor(
                        o[:],
                        pM[:, 0:FT2],
                        _v(dri[:], 0, [[1, 1], [0, D2]]), ALU.mult)
                    nc.vector.tensor_add(o[:], o[:], B2[:])
                    nc.sync.dma_start(outD[w * 128:(w + 1) * 128, :], o[:])

                edge_pass(T1, dm.T1C, dm.G1E, T1, dm.T1C, dm.G1E, dm.ER1U, dstw_sb, FT1, H1, True, TD1, sz1)

                # ---- P3: h' -> ft2ext -> AGin; AllGather -> T2 ----
                for w in range(dm.W):
                    pF = ps_med.tile([128, FT2 + 2], F32, tag="psmed")
                    for fc in range(dm.KC2):
                        pT = ps_big.tile([128, 128], F32, tag="psbig")
                        nc.tensor.transpose(pT[:], h_sb[w][:, fc * 128:(fc + 1) * 128], ident_sb[:])
                        hT = work.tile([128, 128], F32R, tag="hT")
                        nc.vector.tensor_copy(hT[:], pT[:])
                        nc.tensor.matmul(pF[:], hT[:], w2ext[fc][:],
                                         start=(fc == 0), stop=(fc == dm.KC2 - 1))
                    st = spool.tile([128, dm.G2E], TD2, tag="st2")
                    nc.vector.tensor_copy(st[:, 0:FT2], pF[:, 0:FT2])
                    stf2 = st[:].bitcast(F32)
                    nc.vector.tensor_copy(_v(stf2, FT2 * sz2 // 4, [[1, 2]]), pF[:, FT2:FT2 + 2])
                    nc.sync.dma_start(AGin[w * 128:(w + 1) * 128, :], st[:])
                    ste = spool.tile([128, dm.ER2U], TD2, tag="st2e")
                    stef = ste[:].bitcast(F32)
                    nc.vector.tensor_copy(_v(stef, 0, [[1, 1]]), pF[:, FT2 + 1:FT2 + 2])
                    nc.sync.dma_start(ER2loc[w * 128:(w + 1) * 128, :], ste[:])
                if not no_collective:
                    nc.gpsimd.collective_compute(
                        "AllGather", ALU.bypass,
                        replica_groups=[list(range(CORES))],
                        ins=[AGin[:].opt()],
                        outs=[T2[:].opt()])

                # ---- P4: layer-2 edge pass ----
                edge_pass(T2, dm.G2E, dm.G2E, ER2loc, dm.ER2U, 0, dm.ER2U, dstl_sb, FT2, 1, False, TD2, sz2)

    nc.compile()
    return nc


# ---------------- host-side driver ----------------

_CACHE = {}
_last_in_maps = None
_last_dm = None


def _prep_graph(src, dst, dm: Dims):
    """Partition+sort edges by (dst core, window); returns per-core index arrays."""
    E = src.shape[0]
    c_of = dst // dm.NPC
    local = dst - c_of * dm.NPC
    win = local // 128
    rel = (local - win * 128).astype(np.float32)
    srcg = (dm.NPAD * (src // dm.NPC) + (src % dm.NPC)).astype(np.int64)
    order = np.lexsort((np.arange(E), win, c_of))

    counts = np.zeros((CORES, dm.W), np.int64)
    np.add.at(counts, (c_of, win), 1)
    t_w = int(_cdiv(int(counts.max()), 128))
    per_core = []
    epw = t_w * 128
    for c in range(CORES):
        sg = np.zeros(dm.W * epw, np.int64)
        dg = np.zeros(dm.W * epw, np.int64)
        rl = np.full(dm.W * epw, 200.0, np.float32)
        for w in range(dm.W):
            sel = order[(c_of[order] == c) & (win[order] == w)]
            n = sel.shape[0]
            assert n <= epw
            sg[w * epw:w * epw + n] = srcg[sel]
            dg[w * epw:w * epw + n] = dm.NPAD * c + local[sel]
            rl[w * epw:w * epw + n] = rel[sel]
        per_core.append((sg, dg, rl))
    return t_w, per_core


def kernel(x, src, dst, W1, al1, ar1, b1, W2, al2, ar2, b2):
    x = np.asarray(x, np.float32)
    src = np.asarray(src).astype(np.int64)
    dst = np.asarray(dst).astype(np.int64)
    W1 = np.asarray(W1, np.float32)
    W2 = np.asarray(W2, np.float32)
    al1 = np.asarray(al1, np.float32)
    ar1 = np.asarray(ar1, np.float32)
    al2 = np.asarray(al2, np.float32)
    ar2 = np.asarray(ar2, np.float32)
    b1 = np.asarray(b1, np.float32)
    b2 = np.asarray(b2, np.float32)

    N, IN = x.shape
    H1, D1 = al1.shape
    H2, D2 = al2.shape
    assert H2 == 1
    dm0 = Dims(N, IN, H1, D1, D2, 1, CHUNK)
    t_w, per_core = _prep_graph(src, dst, dm0)
    dm = Dims(N, IN, H1, D1, D2, t_w, CHUNK, bf1=BF1, bf2=BF2)

    key = dm.key()
    if key not in _CACHE:
        _CACHE[key] = build_program(dm)
    nc = _CACHE[key]
    global _last_dm
    _last_dm = dm

    # shared host arrays
    xpad = np.zeros((dm.ROWS, IN), np.float32)
    for c in range(CORES):
        xpad[c * dm.NPAD:c * dm.NPAD + dm.NPC] = x[c * dm.NPC:(c + 1) * dm.NPC]
    xT = np.ascontiguousarray(xpad.T)
    AlB1 = np.zeros((dm.FT1, H1), np.float32)
    ArB1 = np.zeros((dm.FT1, H1), np.float32)
    for h in range(H1):
        AlB1[h * D1:(h + 1) * D1, h] = al1[h]
        ArB1[h * D1:(h + 1) * D1, h] = ar1[h]
    shared = {
        "xT": xT,
        "W1": W1, "W1T": np.ascontiguousarray(W1.T),
        "AlB1": AlB1, "ArB1": ArB1, "b1r": b1.reshape(1, -1),
        "W2": W2, "W2T": np.ascontiguousarray(W2.T),
        "al2c": np.ascontiguousarray(al2.reshape(D2, 1)),
        "ar2c": np.ascontiguousarray(ar2.reshape(D2, 1)),
        "b2r": b2.reshape(1, -1),
        "iota": np.tile(np.arange(128, dtype=np.float32), (128, 1)),
        "ident": np.eye(128, dtype=np.float32),
    }
    in_maps = []
    for c in range(CORES):
        sg, dg, rl = per_core[c]
        m = dict(shared)
        m["srcw"] = np.ascontiguousarray(np.tile(sg.reshape(-1, 16).T.astype(np.int16), (8, 1)))
        m["dstw"] = np.ascontiguousarray(np.tile(dg.reshape(-1, 16).T.astype(np.int16), (8, 1)))
        dl = dg - dg // dm.NPAD * dm.NPAD
        m["dstl"] = np.ascontiguousarray(np.tile(dl.reshape(-1, 16).T.astype(np.int16), (8, 1)))
        m["drel"] = np.ascontiguousarray(rl.reshape(dm.T1T, 128).T)
        in_maps.append(m)

    global _last_in_maps
    _last_in_maps = in_maps
    res = run_bass_kernel_spmd(nc, in_maps, core_ids=list(range(CORES)))
    out = np.empty((N, D2), np.float32)
    for c in range(CORES):
        out[c * dm.NPC:(c + 1) * dm.NPC] = res.results[c]["out"][:dm.NPC]
    return out

